# revision 1
# baseline (speedup 1.0000x reference)
"""BertAttention Trainium2 kernel (8 NeuronCores, SPMD, no collectives).

Sharding: DP over batch (2) x sequence-parallel over 512-row query blocks (4).
Each core computes full K/V for its batch (all heads), attention for its
query slice, output projection + residual + LayerNorm for its rows.

Orientation: head-dim on partitions throughout ("transposed"):
  xT = x.T staged via PE transposes (f32r, 1.5 cyc/row)
  qT/kT = W.T @ xT               (per head-pair chunk of 128 rows)
  sT2[128 sk, 1024] = both heads' score tiles side by side (2 PSUM banks)
  p~ = exp(sT2/8 + mask)         (mask is per-partition here -> exp bias)
  ctxT[65, 512] += v_aug[sk_tile].T @ p~half  (col 64 of v_aug = 1.0
                                   -> row 64 of ctxT = softmax denominator)
  ctxT /= denom  (fast reciprocal; K=1 ones-matmul broadcasts across parts)
  o = ctxT.T @ Wo (+bo) + x  -> LayerNorm -> out

Matmuls run in float32r (TF32-like, full PE rate); residual/LN in fp32.
"""

import numpy as np

import bass_rust as _br
import concourse.bass as bass
import concourse.tile as tile
from concourse import mybir
from concourse.bass_utils import run_bass_kernel_spmd
from concourse.masks import make_identity

F32 = mybir.dt.float32
F32R = mybir.dt.float32r

S = 2048
H = 1024
NH = 16
HD = 64
P = 128
SQ = 512          # query rows per core
NKT = S // P      # 16 sk tiles
HC = H // P       # 8 h-chunks
NPAIR = 8         # head pairs (2 heads / 128-row chunk)
EPS = 1e-12

_wait_ctr = [0]


def _split_excess_waits(nc, limit=1):
    """walrus in this container rejects >1-2 sem waits on several opcode
    structs; move excess waits onto same-engine NoOps inserted just before."""
    for f in nc.m.functions:
        for bb in f.blocks:
            insts = bb.instructions
            out = []
            dirty = False
            for inst in insts:
                si = inst.sync_info
                waits = list(si.on_wait) if si and si.on_wait else []
                if len(waits) > limit and inst.engine != mybir.EngineType.Unassigned:
                    for i in range(0, len(waits) - limit, limit):
                        _wait_ctr[0] += 1
                        nop = _br.InstNoOp(
                            name=f"I-waitsplit-{_wait_ctr[0]}", ins=[], outs=[]
                        )
                        nop.engine = inst.engine
                        nop.sync_info = mybir.SyncInfo(
                            on_wait=waits[i : i + limit], on_update=[]
                        )
                        out.append(nop)
                    si.on_wait = waits[len(waits) - limit :]
                    dirty = True
                out.append(inst)
            if dirty:
                bb.instructions = out
    return nc


def build_nc():
    nc = bass.Bass()

    x_d = nc.dram_tensor("x", [S, H], F32R, kind="ExternalInput")
    xq_d = nc.dram_tensor("xq", [SQ, H], F32, kind="ExternalInput")
    # weights pre-tiled on host: [pair, p(row-in-chunk), hc*128+col]
    wq_d = nc.dram_tensor("wq", [NPAIR, P, H], F32R, kind="ExternalInput")
    wk_d = nc.dram_tensor("wk", [NPAIR, P, H], F32R, kind="ExternalInput")
    wv_d = nc.dram_tensor("wv", [NPAIR, P, H], F32R, kind="ExternalInput")
    wo_d = nc.dram_tensor("wo", [H, H], F32R, kind="ExternalInput")
    bqkv_d = nc.dram_tensor("bqkv", [P, 24], F32, kind="ExternalInput")  # q|k|v
    bo_d = nc.dram_tensor("bo", [H], F32, kind="ExternalInput")
    gamma_d = nc.dram_tensor("gamma", [H], F32, kind="ExternalInput")
    beta_d = nc.dram_tensor("beta", [H], F32, kind="ExternalInput")
    mask_d = nc.dram_tensor("mask", [P, NKT], F32, kind="ExternalInput")
    out_d = nc.dram_tensor("out", [SQ, H], F32, kind="ExternalOutput")

    with tile.TileContext(nc) as tc, nc.allow_low_precision(
        reason="f32r tiles feed TensorE; accumulation stays fp32 in PSUM"
    ):
        consts = tc.alloc_tile_pool(name="consts", bufs=1)
        xq_pool = tc.alloc_tile_pool(name="xqn", bufs=1)
        ctxT_pool = tc.alloc_tile_pool(name="ctxT", bufs=1)
        ps_mm = tc.alloc_tile_pool(name="ps_mm", bufs=2, space="PSUM")
        ps_s = tc.alloc_tile_pool(name="ps_s", bufs=2, space="PSUM")
        ps_ctx = tc.alloc_tile_pool(name="ps_ctx", bufs=2, space="PSUM")

        # ---- constants ----
        ident = consts.tile([P, P], F32, tag="ident")
        make_identity(nc, ident)
        ident_r = consts.tile([P, P], F32R, tag="ident_r")
        nc.vector.tensor_copy(ident_r, ident)
        bqkv = consts.tile([P, 24], F32, tag="bqkv")
        nc.sync.dma_start(out=bqkv, in_=bqkv_d[:, :])
        mask = consts.tile([P, NKT], F32, tag="mask")
        nc.sync.dma_start(out=mask, in_=mask_d[:, :])
        bo_bc = consts.tile([P, H], F32, tag="bo_bc")
        nc.sync.dma_start(out=bo_bc, in_=bo_d[:].partition_broadcast(P))
        gamma_bc = consts.tile([P, H], F32, tag="gamma_bc")
        nc.sync.dma_start(out=gamma_bc, in_=gamma_d[:].partition_broadcast(P))
        beta_bc = consts.tile([P, H], F32, tag="beta_bc")
        nc.sync.dma_start(out=beta_bc, in_=beta_d[:].partition_broadcast(P))
        eps_t = consts.tile([P, 1], F32, tag="eps")
        nc.vector.memset(eps_t, EPS)
        ones_f = consts.tile([1, 64], F32, tag="ones_f")
        nc.vector.memset(ones_f, 1.0)

        # ---- phase 1: stage xT (f32r) and xqT via PE transposes ----
        # xT_all[:, c*S + s] = x[s, c*128 + p]; strided copies batch 4 chunks.
        xT_pool = tc.alloc_tile_pool(name="xT", bufs=1)
        xqT_pool = tc.alloc_tile_pool(name="xqT", bufs=1)
        xs_pool = tc.alloc_tile_pool(name="xstream", bufs=3)
        xT_all = xT_pool.tile([P, HC * S], F32R, name="xT_all", tag="xT_all")
        xT_v = xT_all.rearrange("p (c s) -> p c s", c=HC)
        for st in range(NKT):
            xt = xs_pool.tile([P, H], F32R, name="xt", tag="xt")
            nc.sync.dma_start(out=xt, in_=x_d[st * P : (st + 1) * P, :])
            for half in range(2):
                ps = ps_s.tile([P, 1024], F32, name="tr_ps", tag="s2")
                for c4 in range(4):
                    c = half * 4 + c4
                    nc.tensor.transpose(
                        ps[:, c4 * P : (c4 + 1) * P],
                        xt[:, c * P : (c + 1) * P],
                        ident_r,
                    )
                dst = xT_v[:, half * 4 : half * 4 + 4, st * P : (st + 1) * P]
                src = ps.rearrange("p (c s) -> p c s", c=4)
                if st % 2 == 0:
                    nc.vector.tensor_copy(dst, src)
                else:
                    nc.scalar.copy(dst, src)

        xq = [xq_pool.tile([P, H], F32, name=f"xq{i}", tag=f"xq{i}") for i in range(SQ // P)]
        xqT_all = xqT_pool.tile([P, HC * SQ], F32R, name="xqT_all", tag="xqT_all")
        xqT_v = xqT_all.rearrange("p (c s) -> p c s", c=HC)
        for st in range(SQ // P):
            nc.sync.dma_start(out=xq[st], in_=xq_d[st * P : (st + 1) * P, :])
            for half in range(2):
                ps = ps_s.tile([P, 1024], F32, name="tr_ps", tag="s2")
                for c4 in range(4):
                    c = half * 4 + c4
                    nc.tensor.transpose(
                        ps[:, c4 * P : (c4 + 1) * P],
                        xq[st][:, c * P : (c + 1) * P],
                        ident,
                    )
                dst = xqT_v[:, half * 4 : half * 4 + 4, st * P : (st + 1) * P]
                src = ps.rearrange("p (c s) -> p c s", c=4)
                if st % 2 == 0:
                    nc.vector.tensor_copy(dst, src)
                else:
                    nc.scalar.copy(dst, src)

        def xT_c(c):
            return xT_all[:, c * S : (c + 1) * S]

        def xqT_c(c):
            return xqT_all[:, c * SQ : (c + 1) * SQ]

        # ---- phase 2: per head-pair ----
        w_pool = tc.alloc_tile_pool(name="w", bufs=2)
        kv_pool = tc.alloc_tile_pool(name="kv", bufs=1)
        q_pool = tc.alloc_tile_pool(name="q", bufs=2)
        vaug_pool = tc.alloc_tile_pool(name="vaug", bufs=1)
        pt_pool = tc.alloc_tile_pool(name="pt", bufs=3)
        r_pool = tc.alloc_tile_pool(name="r", bufs=2)
        ctxT = [
            ctxT_pool.tile([P, SQ], F32R, name=f"ctxT{m}", tag=f"ctxT{m}")
            for m in range(NPAIR)
        ]
        # persistent v_aug tiles: [d0(64) | 1.0 | d1(64) | 1.0] per sk-tile;
        # ones columns written once, data columns overwritten per pair
        vaug = [
            vaug_pool.tile([P, 130], F32R, name=f"vaug{t}", tag=f"vaug{t}")
            for t in range(NKT)
        ]
        for t in range(NKT):
            ones_view = bass.AP(
                tensor=vaug[t].tensor,
                offset=vaug[t].offset + 64,
                ap=[list(vaug[t].ap[0]), [65, 2], [1, 1]],
            )
            nc.gpsimd.memset(ones_view, 1.0)

        for m in range(NPAIR):
            wq_m = w_pool.tile([P, H], F32R, tag="wq_m")
            nc.sync.dma_start(out=wq_m, in_=wq_d[m])
            wk_m = w_pool.tile([P, H], F32R, tag="wk_m")
            nc.sync.dma_start(out=wk_m, in_=wk_d[m])
            wv_m = w_pool.tile([P, H], F32R, tag="wv_m")
            nc.sync.dma_start(out=wv_m, in_=wv_d[m])

            # qT_m [128, 512]
            qT_m = q_pool.tile([P, SQ], F32R, tag="qT_m")
            ps = ps_mm.tile([P, 512], F32, name="ps", tag="ps")
            for c in range(HC):
                nc.tensor.matmul(
                    ps,
                    wq_m[:, c * P : (c + 1) * P],
                    xqT_c(c),
                    start=(c == 0),
                    stop=(c == HC - 1),
                )
            nc.vector.tensor_scalar_add(qT_m, ps, bqkv[:, m : m + 1])

            # kT_m [128, 2048]
            kT_m = kv_pool.tile([P, S], F32R, tag="kT_m")
            for n in range(S // 512):
                ps = ps_mm.tile([P, 512], F32, name="ps", tag="ps")
                for c in range(HC):
                    nc.tensor.matmul(
                        ps,
                        wk_m[:, c * P : (c + 1) * P],
                        xT_c(c)[:, n * 512 : (n + 1) * 512],
                        start=(c == 0),
                        stop=(c == HC - 1),
                    )
                if n % 2 == 0:
                    nc.vector.tensor_scalar_add(
                        kT_m[:, n * 512 : (n + 1) * 512], ps, bqkv[:, 8 + m : 9 + m]
                    )
                else:
                    nc.scalar.activation(
                        kT_m[:, n * 512 : (n + 1) * 512],
                        ps,
                        mybir.ActivationFunctionType.Identity,
                        bias=bqkv[:, 8 + m : 9 + m],
                    )

            # vT_m [128, 2048], then transpose into v_aug tiles
            vT_m = kv_pool.tile([P, S], F32R, tag="vT_m")
            for n in range(S // 512):
                ps = ps_mm.tile([P, 512], F32, name="ps", tag="ps")
                for c in range(HC):
                    nc.tensor.matmul(
                        ps,
                        wv_m[:, c * P : (c + 1) * P],
                        xT_c(c)[:, n * 512 : (n + 1) * 512],
                        start=(c == 0),
                        stop=(c == HC - 1),
                    )
                if n % 2 == 0:
                    nc.vector.tensor_scalar_add(
                        vT_m[:, n * 512 : (n + 1) * 512], ps, bqkv[:, 16 + m : 17 + m]
                    )
                else:
                    nc.scalar.activation(
                        vT_m[:, n * 512 : (n + 1) * 512],
                        ps,
                        mybir.ActivationFunctionType.Identity,
                        bias=bqkv[:, 16 + m : 17 + m],
                    )

            for t in range(NKT):
                ps = ps_mm.tile([P, 512], F32, name="tr2_ps", tag="ps")
                nc.tensor.transpose(
                    ps[:, 0:128], vT_m[:, t * P : (t + 1) * P], ident_r
                )
                # one strided copy: [0:64] -> va[0:64], [64:128] -> va[65:129]
                dst = bass.AP(
                    tensor=vaug[t].tensor,
                    offset=vaug[t].offset,
                    ap=[list(vaug[t].ap[0]), [65, 2], [1, 64]],
                )
                src = ps[:, 0:128].rearrange("p (two s) -> p two s", two=2)
                if t % 2 == 0:
                    nc.vector.tensor_copy(dst, src)
                else:
                    nc.scalar.copy(dst, src)

            # attention: both heads per sk-tile; score tiles side by side in
            # one [128, 1024] PSUM pair so a single exp covers both heads
            ctx_ps2 = [
                ps_ctx.tile([P, 512], F32, name=f"ctx_ps{h}", tag="ctx_ps")
                for h in range(2)
            ]
            for t in range(NKT):
                s2 = ps_s.tile([P, 1024], F32, name="s2", tag="s2")
                for h in range(2):
                    nc.tensor.matmul(
                        s2[:, h * 512 : (h + 1) * 512],
                        kT_m[64 * h : 64 * h + 64, t * P : (t + 1) * P],
                        qT_m[64 * h : 64 * h + 64, :],
                        start=True,
                        stop=True,
                    )
                pt = pt_pool.tile([P, 1024], F32R, tag="pt", name="pt")
                nc.scalar.activation(
                    pt,
                    s2,
                    mybir.ActivationFunctionType.Exp,
                    scale=0.125,
                    bias=mask[:, t : t + 1],
                )
                for h in range(2):
                    nc.tensor.matmul(
                        ctx_ps2[h][0:65, :],
                        vaug[t][:, 65 * h : 65 * h + 65],
                        pt[:, h * 512 : (h + 1) * 512],
                        start=(t == 0),
                        stop=(t == NKT - 1),
                    )
            for h in range(2):
                # normalize: rows 0..63 /= row 64 (fast reciprocal, then K=1
                # ones-matmul broadcasts it across 64 partitions via PSUM)
                rr = r_pool.tile([1, 512], F32, tag="rr")
                nc.vector.reciprocal_approx_fast(rr, ctx_ps2[h][64:65, :])
                bc_ps = ps_mm.tile([64, 512], F32, tag="ps", name="bc_ps")
                nc.tensor.matmul(bc_ps, ones_f, rr, start=True, stop=True)
                rb = r_pool.tile([64, 512], F32, tag="rb")
                nc.vector.tensor_copy(rb, bc_ps)
                nc.vector.tensor_mul(
                    ctxT[m][64 * h : 64 * h + 64, :], ctx_ps2[h][0:64, :], rb
                )

        for _pool in (r_pool, pt_pool, vaug_pool, q_pool, kv_pool, w_pool,
                      xs_pool, xqT_pool, xT_pool):
            _pool.release()

        # ---- phase 3: output projection + residual + LayerNorm ----
        wo_pool = tc.alloc_tile_pool(name="wo", bufs=1)
        ln_pool = tc.alloc_tile_pool(name="ln", bufs=2)
        wo_sb = [wo_pool.tile([P, H], F32R, name=f"wo{c}", tag=f"wo{c}") for c in range(HC)]
        for c in range(HC):
            nc.sync.dma_start(out=wo_sb[c], in_=wo_d[c * P : (c + 1) * P, :])

        for st in range(SQ // P):
            h_sb = ln_pool.tile([P, H], F32, tag="h_sb")
            for nch in range(2):
                ps = ps_mm.tile([P, 512], F32, name="ps", tag="ps")
                for c in range(HC):
                    nc.tensor.matmul(
                        ps,
                        ctxT[c][:, st * P : (st + 1) * P],
                        wo_sb[c][:, nch * 512 : (nch + 1) * 512],
                        start=(c == 0),
                        stop=(c == HC - 1),
                    )
                nc.vector.tensor_add(
                    h_sb[:, nch * 512 : (nch + 1) * 512],
                    ps,
                    xq[st][:, nch * 512 : (nch + 1) * 512],
                )
            nc.gpsimd.tensor_add(h_sb, h_sb, bo_bc)

            # LayerNorm (biased variance)
            stats = ln_pool.tile([P, 2, 6], F32, tag="stats")
            for g in range(2):
                nc.vector.bn_stats(
                    out=stats[:, g, :], in_=h_sb[:, g * 512 : (g + 1) * 512]
                )
            mv = ln_pool.tile([P, 2], F32, tag="mv")
            nc.vector.bn_aggr(out=mv, in_=stats)
            sd = ln_pool.tile([P, 1], F32, tag="sd")
            nc.scalar.activation(
                sd, mv[:, 1:2], mybir.ActivationFunctionType.Sqrt, bias=eps_t
            )
            rs = ln_pool.tile([P, 1], F32, tag="rs")
            nc.vector.reciprocal(rs, sd)
            xh = ln_pool.tile([P, H], F32, tag="xh")
            nc.vector.tensor_scalar(
                out=xh,
                in0=h_sb,
                scalar1=mv[:, 0:1],
                scalar2=rs,
                op0=mybir.AluOpType.subtract,
                op1=mybir.AluOpType.mult,
            )
            og = ln_pool.tile([P, H], F32, tag="og")
            nc.gpsimd.tensor_mul(og, xh, gamma_bc)
            ob = ln_pool.tile([P, H], F32, tag="ob")
            nc.gpsimd.tensor_add(ob, og, beta_bc)
            nc.sync.dma_start(out=out_d[st * P : (st + 1) * P, :], in_=ob)

        for _pool in (ln_pool, wo_pool, ps_ctx, ps_s, ps_mm, ctxT_pool,
                      xq_pool, consts):
            _pool.release()

    _split_excess_waits(nc)
    return nc


_NC = None


def _get_nc():
    global _NC
    if _NC is None:
        _NC = build_nc()
    return _NC


def _in_maps(hidden_states, attention_mask, Wq, bq, Wk, bk, Wv, bv, Wo, bo, gamma, beta):
    hs = np.ascontiguousarray(np.asarray(hidden_states, dtype=np.float32))
    am = np.asarray(attention_mask, dtype=np.float32).reshape(2, S)

    def tile_w(w):
        w = np.asarray(w, dtype=np.float32)
        # [hc, p, m, col] -> [m, p, hc, col] -> [pair, 128, 1024]
        return np.ascontiguousarray(
            w.reshape(HC, P, NPAIR, P).transpose(2, 1, 0, 3).reshape(NPAIR, P, H)
        )

    wq_t, wk_t, wv_t = tile_w(Wq), tile_w(Wk), tile_w(Wv)
    wo_c = np.ascontiguousarray(np.asarray(Wo, dtype=np.float32))
    bqkv = np.ascontiguousarray(
        np.concatenate(
            [np.asarray(b, dtype=np.float32).reshape(NPAIR, P).T for b in (bq, bk, bv)],
            axis=1,
        )
    )
    bo_c = np.ascontiguousarray(np.asarray(bo, dtype=np.float32))
    g_c = np.ascontiguousarray(np.asarray(gamma, dtype=np.float32))
    be_c = np.ascontiguousarray(np.asarray(beta, dtype=np.float32))

    maps = []
    for core in range(8):
        b, j = core // 4, core % 4
        maps.append(
            {
                "x": hs[b],
                "xq": np.ascontiguousarray(hs[b, j * SQ : (j + 1) * SQ, :]),
                "wq": wq_t,
                "wk": wk_t,
                "wv": wv_t,
                "wo": wo_c,
                "bqkv": bqkv,
                "bo": bo_c,
                "gamma": g_c,
                "beta": be_c,
                "mask": np.ascontiguousarray(am[b].reshape(NKT, P).T),
            }
        )
    return maps


def run(trace=False, **inputs):
    nc = _get_nc()
    maps = _in_maps(**inputs)
    res = run_bass_kernel_spmd(nc, maps, core_ids=list(range(8)), trace=trace)
    out = np.empty((2, S, H), dtype=np.float32)
    for core in range(8):
        b, j = core // 4, core % 4
        out[b, j * SQ : (j + 1) * SQ, :] = res.results[core]["out"]
    return out, res


def kernel(**inputs):
    out, _ = run(trace=False, **inputs)
    return out
